# revision 9
# baseline (speedup 1.0000x reference)
"""Causal self-attention (B=2, T=2048, C=2048, H=16) on 8 TRN2 NeuronCores.

Sharding: data-parallel over batch (2) x tensor-parallel over heads (4 heads
per core). Each core computes its 4 heads' QKV projection, causal attention,
and a partial output projection (row-sharded W_proj); the host sums the 4
partial projections per batch element.

Key speed structure vs the fp16 baseline:
  * QKV projection runs in error-compensated fp8 with DoubleRow perf mode
    (2 c-tiles contracted per pass): x@W ~= x8@W8 + dx8@W8 + (x/16)8@(16dW)8.
    All three operand tensors come precomputed from the host, so the 3-term
    sum costs 0.75x the fp16 matmul time at fp16-level accuracy.
  * Attention keeps S^T = K^T.T@Q^T (fp16) but the AV matmul is flipped:
    stationary = V block [k,d], moving = P^T [k, 512q] -> Y^T [d, q] lands in
    PSUM directly in the layout the projection consumes. This removes the 64
    PE transposes and the 129-column LDWEIGHTS-bound AV matmuls.
  * Softmax denominator: P^T tiles are also accumulated (fp16, DVE) into
    ptsum[k, q]; a single ones-vector matmul gives denom[1, q], DVE takes the
    reciprocal, and a rank-1 ones x recip matmul broadcasts it to [128, q];
    one DVE multiply normalizes Y^T while converting to fp16.
  * Projection outputs DMA straight from PSUM to DRAM (no PSUM->SBUF copy),
    and proj matmul groups of chunk j-1 are interleaved into the attention
    chains of chunk j to fill the PE while ACT works on exp.
"""

import os

import numpy as np
import ml_dtypes

N_HEAD = 16
N_EMBD = 2048
B = 2
T = 2048
C = N_EMBD
D = C // N_HEAD  # 128
HPC = N_HEAD // 4  # heads per core = 4
N_CORES = 8
CT = C // 128  # 16 contraction tiles
TT = T // 128  # 16 t tiles
NCH = T // 512  # 4 chunks of 512

LAST_EXEC_NS = None

_CACHE = {}


def _build_nc():
    import concourse.bass as bass  # noqa: F401
    import concourse.tile as tile
    from concourse import bacc, mybir

    F32 = mybir.dt.float32
    F16 = mybir.dt.float16
    F8 = mybir.dt.float8e4
    Exp = mybir.ActivationFunctionType.Exp
    Copy = mybir.ActivationFunctionType.Copy
    DR = mybir.MatmulPerfMode.DoubleRow
    SCALE = 1.0 / float(np.sqrt(D))

    nc = bacc.Bacc("TRN2", target_bir_lowering=False, num_devices=N_CORES)

    x8_d = nc.dram_tensor("x8", [C, T], F8, kind="ExternalInput")
    dx8_d = nc.dram_tensor("dx8", [C, T], F8, kind="ExternalInput")
    xb8_d = nc.dram_tensor("xb8", [C, T], F8, kind="ExternalInput")
    wqk8_d = nc.dram_tensor("wqk8", [C, 8 * 128], F8, kind="ExternalInput")
    dwqk8_d = nc.dram_tensor("dwqk8", [C, 8 * 128], F8, kind="ExternalInput")
    wv8_d = nc.dram_tensor("wv8", [C, 4 * 128], F8, kind="ExternalInput")
    dwv8_d = nc.dram_tensor("dwv8", [C, 4 * 128], F8, kind="ExternalInput")
    wp_d = nc.dram_tensor("wp", [4 * 128, C], F16, kind="ExternalInput")
    out_d = nc.dram_tensor("out_part", [T, C], F16, kind="ExternalOutput")

    # Constants baked into the NEFF: diagonal causal masks + ones vectors.
    kk = np.arange(128)[:, None]
    qq = np.arange(512)[None, :]
    masks = np.stack(
        [(qq >= (128 * i + kk)).astype(np.float16) for i in range(4)]
    )  # [4, 128, 512]
    masks_d = nc.inline_tensor(np.ascontiguousarray(masks), name="diagmasks")
    ones128_d = nc.inline_tensor(np.ones((128, 1), dtype=np.float16), name="ones128")

    with tile.TileContext(nc) as tc:
        with (
            tc.tile_pool(name="singles", bufs=1) as singles,
            tc.tile_pool(name="xtp", bufs=6) as xtp,
            tc.tile_pool(name="ptp", bufs=6) as ptp,
            tc.tile_pool(name="ptsp", bufs=3) as ptsp,
            tc.tile_pool(name="rtp", bufs=3) as rtp,
            tc.tile_pool(name="ost", bufs=3) as ostp,
            tc.tile_pool(name="rbp", bufs=2) as rbp,
            tc.tile_pool(name="ps", bufs=4, space="PSUM") as ps,
            tc.tile_pool(name="pop", bufs=2, space="PSUM") as pop,
            tc.tile_pool(name="yps", bufs=2, space="PSUM") as yps,
        ):
            # ---- input loads ----
            def load_x_chunk(tj):
                tri = []
                for nm, dt_ in (("a", x8_d), ("b", dx8_d), ("c", xb8_d)):
                    xc = xtp.tile([128, CT, 512], F8, tag="xt", name=f"x{nm}{tj}")
                    nc.sync.dma_start(
                        out=xc,
                        in_=dt_[:, tj * 512 : (tj + 1) * 512].rearrange(
                            "(a p) n -> p a n", p=128
                        ),
                    )
                    tri.append(xc)
                return tri

            xt0 = load_x_chunk(0)
            # QK weights in per-coltile slices so the first matmul only waits
            # on ~256 KB.
            wqk_t = []
            dwqk_t = []
            for ct in range(8):
                w = singles.tile([128, CT, 128], F8, name=f"wqk{ct}")
                nc.sync.dma_start(
                    out=w,
                    in_=wqk8_d[:, ct * 128 : (ct + 1) * 128].rearrange(
                        "(a p) n -> p a n", p=128
                    ),
                )
                wqk_t.append(w)
                w = singles.tile([128, CT, 128], F8, name=f"dwqk{ct}")
                nc.sync.dma_start(
                    out=w,
                    in_=dwqk8_d[:, ct * 128 : (ct + 1) * 128].rearrange(
                        "(a p) n -> p a n", p=128
                    ),
                )
                dwqk_t.append(w)

            # qkt: [d, coltile, t]; coltiles 0..3 = Q heads, 4..7 = K heads
            qkt_sb = singles.tile([128, 8, T], F16)
            # v: [kt-tile, head, d]
            vv_sb = singles.tile([128, TT, HPC, 128], F16)
            # y transposed: [d, head, t]
            yt_sb = singles.tile([128, HPC, T], F16)
            wv_sb = None
            dwv_sb = None
            wp_sb = None
            mask_sb = None
            ones128_sb = None

            # ---- Phase 1: QKV projection (compensated fp8, DoubleRow) ----
            for tj in range(NCH):
                xt = xt0 if tj == 0 else load_x_chunk(tj)
                xa, xb, xc = xt
                for ct in range(8):
                    pq = ps.tile([128, 512], F32, tag="ps", name=f"pq{tj}_{ct}")
                    terms = [(xa, wqk_t[ct]), (xb, wqk_t[ct]), (xc, dwqk_t[ct])]
                    for ti, (xop, wop) in enumerate(terms):
                        for cp in range(CT // 2):
                            nc.tensor.matmul(
                                pq,
                                wop[:, 2 * cp : 2 * cp + 2, :],
                                xop[:, 2 * cp : 2 * cp + 2, :],
                                start=(ti == 0 and cp == 0),
                                stop=(ti == 2 and cp == CT // 2 - 1),
                                perf_mode=DR,
                            )
                    nc.scalar.activation(
                        out=qkt_sb[:, ct, tj * 512 : (tj + 1) * 512],
                        in_=pq,
                        func=Copy,
                    )
                if tj == 0:
                    wv_sb = singles.tile([128, CT, 512], F8, name="wv_sb")
                    nc.sync.dma_start(
                        out=wv_sb, in_=wv8_d[:, :].rearrange("(a p) n -> p a n", p=128)
                    )
                    dwv_sb = singles.tile([128, CT, 512], F8, name="dwv_sb")
                    nc.sync.dma_start(
                        out=dwv_sb,
                        in_=dwv8_d[:, :].rearrange("(a p) n -> p a n", p=128),
                    )
                for tt in range(4):
                    kt = tj * 4 + tt
                    pv = ps.tile([128, 512], F32, tag="ps", name=f"pv{kt}")
                    terms = [(xa, wv_sb), (xb, wv_sb), (xc, dwv_sb)]
                    for ti, (xop, wop) in enumerate(terms):
                        for cp in range(CT // 2):
                            nc.tensor.matmul(
                                pv,
                                xop[:, 2 * cp : 2 * cp + 2, tt * 128 : (tt + 1) * 128],
                                wop[:, 2 * cp : 2 * cp + 2, :],
                                start=(ti == 0 and cp == 0),
                                stop=(ti == 2 and cp == CT // 2 - 1),
                                perf_mode=DR,
                            )
                    nc.scalar.activation(
                        out=vv_sb[:, kt, :, :],
                        in_=pv.rearrange("p (h d) -> p h d", h=HPC),
                        func=Copy,
                    )
                if tj == 0:
                    # First needed by attention / projection; loaded during
                    # phase 1.
                    wp_sb = singles.tile([128, HPC, C], F16, name="wp_sb")
                    nc.sync.dma_start(
                        out=wp_sb,
                        in_=wp_d[:, :].rearrange("(a p) n -> p a n", p=128),
                    )
                    mask_sb = singles.tile([128, 4, 512], F16, name="mask_sb")
                    nc.sync.dma_start(
                        out=mask_sb, in_=masks_d[:, :, :].rearrange("a p n -> p a n")
                    )
                    ones128_sb = singles.tile([128, 1], F16, name="ones128_sb")
                    nc.sync.dma_start(out=ones128_sb, in_=ones128_d[:, :])

            # ---- Phases 2+3: attention + projection, interleaved ----
            ot_cur = {}

            def emit_proj_group(tt, cc):
                po = pop.tile([128, 512], F32, tag="po", name=f"po{tt}_{cc}")
                for hd in range(HPC):
                    nc.tensor.matmul(
                        po,
                        yt_sb[:, hd, tt * 128 : (tt + 1) * 128],
                        wp_sb[:, hd, cc * 512 : (cc + 1) * 512],
                        start=(hd == 0),
                        stop=(hd == HPC - 1),
                    )
                if cc == 0:
                    ot_cur[tt] = ostp.tile([128, C], F16, tag="ot", name=f"ot{tt}")
                ot = ot_cur[tt]
                if cc % 2 == 0:
                    nc.vector.tensor_copy(out=ot[:, cc * 512 : (cc + 1) * 512], in_=po)
                else:
                    nc.scalar.activation(
                        out=ot[:, cc * 512 : (cc + 1) * 512], in_=po, func=Copy
                    )
                if cc == 3:
                    nc.sync.dma_start(out=out_d[tt * 128 : (tt + 1) * 128, :], in_=ot)

            prev_proj = []
            for j in range(NCH):
                pq_iter = iter(prev_proj)
                nkt = 4 * j + 4
                for h in range(HPC):
                    yT = yps.tile([128, 512], F32, tag="y", name=f"yT{h}_{j}")
                    pts = ptsp.tile([128, 512], F16, tag="pts", name=f"pts{h}_{j}")
                    for kt in range(nkt):
                        di = kt - 4 * j
                        lo = 128 * di if di > 0 else 0
                        ss = ps.tile([128, 512], F32, tag="ps", name=f"ss{h}{j}{kt}")
                        nc.tensor.matmul(
                            ss[:, lo:],
                            qkt_sb[:, 4 + h, kt * 128 : (kt + 1) * 128],
                            qkt_sb[:, h, j * 512 + lo : (j + 1) * 512],
                            start=True,
                            stop=True,
                        )
                        pt = ptp.tile([128, 512], F16, tag="pt", name=f"pt{h}{j}{kt}")
                        nc.scalar.activation(
                            out=pt[:, lo:], in_=ss[:, lo:], func=Exp, scale=SCALE
                        )
                        if di >= 0:
                            nc.vector.tensor_mul(
                                pt[:, lo : lo + 128],
                                pt[:, lo : lo + 128],
                                mask_sb[:, di, lo : lo + 128],
                            )
                            if lo > 0:
                                nc.vector.memset(pt[:, 0:lo], 0.0)
                        if kt == 0:
                            nc.vector.tensor_copy(out=pts, in_=pt)
                        else:
                            nc.vector.tensor_add(pts, pts, pt)
                        nc.tensor.matmul(
                            yT,
                            vv_sb[:, kt, h, :],
                            pt,
                            start=(kt == 0),
                            stop=(kt == nkt - 1),
                        )
                        if kt % 2 == 1:
                            nxt = next(pq_iter, None)
                            if nxt is not None:
                                emit_proj_group(*nxt)
                    # softmax denominator -> broadcast reciprocal -> normalize
                    dT = ps.tile([1, 512], F32, tag="ps", name=f"dT{h}_{j}")
                    nc.tensor.matmul(dT, ones128_sb, pts, start=True, stop=True)
                    rT = rtp.tile([1, 512], F16, tag="rt", name=f"rT{h}_{j}")
                    with nc.allow_low_precision("fp16 softmax recip is plenty"):
                        nc.vector.reciprocal(rT, dT)
                    rb = rbp.tile([128, 512], F16, tag="rb", name=f"rb{h}_{j}")
                    nc.gpsimd.partition_broadcast(rb, rT)
                    nc.vector.tensor_mul(
                        yt_sb[:, h, j * 512 : (j + 1) * 512], yT, rb
                    )
                for rem in pq_iter:
                    emit_proj_group(*rem)
                prev_proj = [(tt, cc) for tt in range(4 * j, 4 * j + 4) for cc in range(4)]
            for tt, cc in prev_proj:
                emit_proj_group(tt, cc)

    nc.compile()
    return nc


def _get_nc():
    if "nc" not in _CACHE:
        _CACHE["nc"] = _build_nc()
    return _CACHE["nc"]


F8NP = ml_dtypes.float8_e4m3


def _q8(a):
    return a.astype(F8NP)


def kernel(x, W_attn, W_proj):
    global LAST_EXEC_NS
    from concourse.bass_utils import run_bass_kernel_spmd

    x = np.asarray(x, dtype=np.float32)
    W_attn = np.asarray(W_attn, dtype=np.float32)
    W_proj = np.asarray(W_proj, dtype=np.float32)

    # Per-batch x tensors (shared by the 4 cores of each batch element).
    xprep = []
    for b in range(B):
        xT = np.ascontiguousarray(x[b].T)
        x8 = _q8(xT)
        dx8 = _q8(xT - x8.astype(np.float32))
        xb8 = _q8(xT * (1.0 / 16.0))
        xprep.append((x8, dx8, xb8))

    in_maps = []
    for core in range(N_CORES):
        b, g = divmod(core, 4)
        heads = range(4 * g, 4 * g + 4)
        x8, dx8, xb8 = xprep[b]
        wqk = np.concatenate(
            [W_attn[:, h * D : (h + 1) * D] for h in heads]
            + [W_attn[:, C + h * D : C + (h + 1) * D] for h in heads],
            axis=1,
        )
        wv = np.concatenate(
            [W_attn[:, 2 * C + h * D : 2 * C + (h + 1) * D] for h in heads], axis=1
        )
        wqk8 = _q8(wqk)
        dwqk8 = _q8(16.0 * (wqk - wqk8.astype(np.float32)))
        wv8 = _q8(wv)
        dwv8 = _q8(16.0 * (wv - wv8.astype(np.float32)))
        wp = W_proj[4 * g * D : 4 * (g + 1) * D, :].astype(np.float16)
        in_maps.append(
            {
                "x8": x8,
                "dx8": dx8,
                "xb8": xb8,
                "wqk8": wqk8,
                "dwqk8": dwqk8,
                "wv8": wv8,
                "dwv8": dwv8,
                "wp": wp,
            }
        )

    nc = _get_nc()
    res = run_bass_kernel_spmd(
        nc,
        in_maps,
        list(range(N_CORES)),
        trace=bool(os.environ.get("KERNEL_TRACE")),
    )
    LAST_EXEC_NS = res.exec_time_ns

    out = np.zeros((B, T, C), dtype=np.float32)
    for core in range(N_CORES):
        b = core // 4
        out[b] += res.results[core]["out_part"].astype(np.float32)
    return out


# revision 12
# speedup vs baseline: 1.2594x; 1.2594x over previous
"""Causal self-attention (B=2, T=2048, C=2048, H=16) on 8 TRN2 NeuronCores.

Sharding: data-parallel over batch (2) x tensor-parallel over heads (4 heads
per core). Each core computes its 4 heads' QKV projection, causal attention,
and a partial output projection (row-sharded W_proj); the host sums the 4
fp16 partial projections per batch element.

All matmuls are fp16 (fp8 DoubleRow on this hardware doubles contraction per
column, not columns per cycle, so any >=2-term compensated-fp8 scheme is no
faster than fp16 while 1-term fp8 misses the accuracy gate). Speed comes from
keeping the PE streaming 512-wide fp16 matmuls continuously:

  * AV is flipped vs the original: stationary = V block [k,d], moving =
    P^T [k, 512q] -> Y^T [d, q] accumulates in PSUM already transposed for
    the projection. No PE transposes, no narrow LDWEIGHTS-bound matmuls.
  * Softmax denominator: P^T tiles accumulate (fp16, DVE) into ptsum[k, q];
    one ones-vector matmul gives denom[1, q]; GpSimd broadcasts it to
    [128, q]; DVE reciprocal + one multiply normalize Y^T into fp16.
  * Software pipelining: the QKV-projection matmul groups of chunk j+1 and
    the output-projection groups of chunk j-1 are interleaved into the
    attention chains of chunk j, so the ACT-bound exp stream always has PE
    work running underneath it and the kernel stays PE-bound end to end.
"""

import os

import numpy as np

N_HEAD = 16
N_EMBD = 2048
B = 2
T = 2048
C = N_EMBD
D = C // N_HEAD  # 128
HPC = N_HEAD // 4  # heads per core = 4
N_CORES = 8
CT = C // 128  # 16 contraction tiles
TT = T // 128  # 16 t tiles
NCH = T // 512  # 4 chunks of 512

LAST_EXEC_NS = None

_CACHE = {}


def _build_nc():
    import concourse.bass as bass  # noqa: F401
    import concourse.tile as tile
    from concourse import bacc, mybir

    F32 = mybir.dt.float32
    F16 = mybir.dt.float16
    Exp = mybir.ActivationFunctionType.Exp
    Copy = mybir.ActivationFunctionType.Copy
    SCALE = 1.0 / float(np.sqrt(D))

    nc = bacc.Bacc("TRN2", target_bir_lowering=False, num_devices=N_CORES)

    xT_d = nc.dram_tensor("xT", [C, T], F16, kind="ExternalInput")
    wqk_d = nc.dram_tensor("wqk", [C, 8 * 128], F16, kind="ExternalInput")
    wv_d = nc.dram_tensor("wv", [C, 4 * 128], F16, kind="ExternalInput")
    wp_d = nc.dram_tensor("wp", [4 * 128, C], F16, kind="ExternalInput")
    out_d = nc.dram_tensor("out_part", [T, C], F16, kind="ExternalOutput")

    kk = np.arange(128)[:, None]
    qq = np.arange(512)[None, :]
    masks = np.stack(
        [(qq >= (128 * i + kk)).astype(np.float16) for i in range(4)]
    )  # [4, 128, 512]
    masks_d = nc.inline_tensor(np.ascontiguousarray(masks), name="diagmasks")
    ones128_d = nc.inline_tensor(np.ones((128, 1), dtype=np.float16), name="ones128")

    with tile.TileContext(nc) as tc:
        with (
            tc.tile_pool(name="singles", bufs=1) as singles,
            tc.tile_pool(name="xtp", bufs=32) as xtp,
            tc.tile_pool(name="ptp", bufs=6) as ptp,
            tc.tile_pool(name="ptsp", bufs=3) as ptsp,
            tc.tile_pool(name="rtp", bufs=2) as rtp,
            tc.tile_pool(name="rbp", bufs=2) as rbp,
            tc.tile_pool(name="ost", bufs=3) as ostp,
            tc.tile_pool(name="ps", bufs=4, space="PSUM") as ps,
            tc.tile_pool(name="pop", bufs=2, space="PSUM") as pop,
            tc.tile_pool(name="yps", bufs=2, space="PSUM") as yps,
        ):
            def load_x_chunk(tj):
                xt = []
                for c in range(CT):
                    xc = xtp.tile([128, 512], F16, tag="xt", name=f"xt{tj}_{c}")
                    nc.sync.dma_start(
                        out=xc,
                        in_=xT_d[
                            c * 128 : (c + 1) * 128, tj * 512 : (tj + 1) * 512
                        ],
                    )
                    xt.append(xc)
                return xt

            xt0 = load_x_chunk(0)
            # QK weights sliced per output col-tile: the first matmul group
            # only waits on one 256 KB tile (+ the chunk-0 x tiles).
            wqk_t = []
            for ct in range(8):
                w = singles.tile([128, CT, 128], F16, name=f"wqk{ct}")
                nc.sync.dma_start(
                    out=w,
                    in_=wqk_d[:, ct * 128 : (ct + 1) * 128].rearrange(
                        "(a p) n -> p a n", p=128
                    ),
                )
                wqk_t.append(w)

            # qkt: [d, coltile, t]; coltiles 0..3 = Q heads, 4..7 = K heads
            qkt_sb = singles.tile([128, 8, T], F16)
            # v: [kt-tile, head, d]
            vv_sb = singles.tile([128, TT, HPC, 128], F16)
            # y transposed: [d, head, t]
            yt_sb = singles.tile([128, HPC, T], F16)

            wv_sb = singles.tile([128, CT, 512], F16, name="wv_sb")
            nc.sync.dma_start(
                out=wv_sb, in_=wv_d[:, :].rearrange("(a p) n -> p a n", p=128)
            )
            wp_sb = singles.tile([128, HPC, C], F16, name="wp_sb")
            nc.sync.dma_start(
                out=wp_sb, in_=wp_d[:, :].rearrange("(a p) n -> p a n", p=128)
            )
            mask_sb = singles.tile([128, 4, 512], F16, name="mask_sb")
            nc.sync.dma_start(
                out=mask_sb, in_=masks_d[:, :, :].rearrange("a p n -> p a n")
            )
            ones128_sb = singles.tile([128, 1], F16, name="ones128_sb")
            nc.sync.dma_start(out=ones128_sb, in_=ones128_d[:, :])

            # ---- emission pieces ----
            def qkv_pieces(tj, xt):
                pieces = []

                def mk_qk(ct):
                    def go():
                        pq = ps.tile([128, 512], F32, tag="ps", name=f"pq{tj}_{ct}")
                        for c in range(CT):
                            nc.tensor.matmul(
                                pq,
                                wqk_t[ct][:, c, :],
                                xt[c],
                                start=(c == 0),
                                stop=(c == CT - 1),
                            )
                        nc.scalar.activation(
                            out=qkt_sb[:, ct, tj * 512 : (tj + 1) * 512],
                            in_=pq,
                            func=Copy,
                        )

                    return go

                def mk_v(tt):
                    def go():
                        kt = tj * 4 + tt
                        pv = ps.tile([128, 512], F32, tag="ps", name=f"pv{kt}")
                        for c in range(CT):
                            nc.tensor.matmul(
                                pv,
                                xt[c][:, tt * 128 : (tt + 1) * 128],
                                wv_sb[:, c, :],
                                start=(c == 0),
                                stop=(c == CT - 1),
                            )
                        nc.scalar.activation(
                            out=vv_sb[:, kt, :, :],
                            in_=pv.rearrange("p (h d) -> p h d", h=HPC),
                            func=Copy,
                        )

                    return go

                for ct in range(8):
                    pieces.append(mk_qk(ct))
                for tt in range(4):
                    pieces.append(mk_v(tt))
                return pieces

            ot_cur = {}

            def proj_pieces(j):
                pieces = []

                def mk(tt, cc):
                    def go():
                        po = pop.tile([128, 512], F32, tag="po", name=f"po{tt}_{cc}")
                        for hd in range(HPC):
                            nc.tensor.matmul(
                                po,
                                yt_sb[:, hd, tt * 128 : (tt + 1) * 128],
                                wp_sb[:, hd, cc * 512 : (cc + 1) * 512],
                                start=(hd == 0),
                                stop=(hd == HPC - 1),
                            )
                        if cc == 0:
                            ot_cur[tt] = ostp.tile(
                                [128, C], F16, tag="ot", name=f"ot{tt}"
                            )
                        ot = ot_cur[tt]
                        if cc % 2 == 0:
                            nc.vector.tensor_copy(
                                out=ot[:, cc * 512 : (cc + 1) * 512], in_=po
                            )
                        else:
                            nc.scalar.activation(
                                out=ot[:, cc * 512 : (cc + 1) * 512], in_=po, func=Copy
                            )
                        if cc == 3:
                            nc.sync.dma_start(
                                out=out_d[tt * 128 : (tt + 1) * 128, :], in_=ot
                            )

                    return go

                for tt in range(4 * j, 4 * j + 4):
                    for cc in range(4):
                        pieces.append(mk(tt, cc))
                return pieces

            # ---- pipelined emission ----
            for p in qkv_pieces(0, xt0):
                p()

            prev_proj = []
            for j in range(NCH):
                if j < NCH - 1:
                    xt_next = load_x_chunk(j + 1)
                    nxt_pieces = qkv_pieces(j + 1, xt_next)
                else:
                    nxt_pieces = []
                # round-robin: proj of chunk j-1 and qkv of chunk j+1
                mixed = []
                a, b = iter(prev_proj), iter(nxt_pieces)
                while True:
                    pa, pb = next(a, None), next(b, None)
                    if pa is None and pb is None:
                        break
                    if pa is not None:
                        mixed.append(pa)
                    if pb is not None:
                        mixed.append(pb)
                filler = iter(mixed)

                nkt = 4 * j + 4
                for h in range(HPC):
                    yT = yps.tile([128, 512], F32, tag="y", name=f"yT{h}_{j}")
                    pts = ptsp.tile([128, 512], F16, tag="pts", name=f"pts{h}_{j}")
                    for kt in range(nkt):
                        di = kt - 4 * j
                        lo = 128 * di if di > 0 else 0
                        ss = ps.tile([128, 512], F32, tag="ps", name=f"ss{h}{j}{kt}")
                        nc.tensor.matmul(
                            ss[:, lo:],
                            qkt_sb[:, 4 + h, kt * 128 : (kt + 1) * 128],
                            qkt_sb[:, h, j * 512 + lo : (j + 1) * 512],
                            start=True,
                            stop=True,
                        )
                        pt = ptp.tile([128, 512], F16, tag="pt", name=f"pt{h}{j}{kt}")
                        nc.scalar.activation(
                            out=pt[:, lo:], in_=ss[:, lo:], func=Exp, scale=SCALE
                        )
                        if di >= 0:
                            nc.vector.tensor_mul(
                                pt[:, lo : lo + 128],
                                pt[:, lo : lo + 128],
                                mask_sb[:, di, lo : lo + 128],
                            )
                        if kt == 0:
                            nc.vector.tensor_copy(out=pts, in_=pt)
                        else:
                            nc.vector.tensor_add(
                                pts[:, lo:], pts[:, lo:], pt[:, lo:]
                            )
                        nc.tensor.matmul(
                            yT[:, lo:],
                            vv_sb[:, kt, h, :],
                            pt[:, lo:],
                            start=(kt == 0),
                            stop=(kt == nkt - 1),
                            skip_group_check=True,
                        )
                        nf = next(filler, None)
                        if nf is not None:
                            nf()
                    # denominator -> broadcast -> reciprocal -> normalize
                    dT = ps.tile([1, 512], F32, tag="ps", name=f"dT{h}_{j}")
                    nc.tensor.matmul(dT, ones128_sb, pts, start=True, stop=True)
                    dTs = rtp.tile([1, 512], F32, tag="dts", name=f"dTs{h}_{j}")
                    nc.scalar.activation(out=dTs, in_=dT, func=Copy)
                    rb32 = rbp.tile([128, 512], F32, tag="rb32", name=f"rb32_{h}{j}")
                    nc.gpsimd.partition_broadcast(rb32, dTs)
                    rb16 = rbp.tile([128, 512], F16, tag="rb16", name=f"rb16_{h}{j}")
                    with nc.allow_low_precision("fp16 softmax recip is plenty"):
                        nc.vector.reciprocal(rb16, rb32)
                    nc.vector.tensor_mul(
                        yt_sb[:, h, j * 512 : (j + 1) * 512], yT, rb16
                    )
                for rem in filler:
                    rem()
                prev_proj = proj_pieces(j)
            for p in prev_proj:
                p()

    nc.compile()
    return nc


def _get_nc():
    if "nc" not in _CACHE:
        _CACHE["nc"] = _build_nc()
    return _CACHE["nc"]


def kernel(x, W_attn, W_proj):
    global LAST_EXEC_NS
    from concourse.bass_utils import run_bass_kernel_spmd

    x = np.asarray(x)
    W_attn = np.asarray(W_attn)
    W_proj = np.asarray(W_proj)

    in_maps = []
    for core in range(N_CORES):
        b, g = divmod(core, 4)
        heads = range(4 * g, 4 * g + 4)
        xT = np.ascontiguousarray(x[b].T).astype(np.float16)
        wqk = np.concatenate(
            [W_attn[:, h * D : (h + 1) * D] for h in heads]
            + [W_attn[:, C + h * D : C + (h + 1) * D] for h in heads],
            axis=1,
        ).astype(np.float16)
        wv = np.concatenate(
            [W_attn[:, 2 * C + h * D : 2 * C + (h + 1) * D] for h in heads], axis=1
        ).astype(np.float16)
        wp = W_proj[4 * g * D : 4 * (g + 1) * D, :].astype(np.float16)
        in_maps.append({"xT": xT, "wqk": wqk, "wv": wv, "wp": wp})

    nc = _get_nc()
    res = run_bass_kernel_spmd(
        nc,
        in_maps,
        list(range(N_CORES)),
        trace=bool(os.environ.get("KERNEL_TRACE")),
    )
    LAST_EXEC_NS = res.exec_time_ns

    out = np.zeros((B, T, C), dtype=np.float32)
    for core in range(N_CORES):
        b = core // 4
        out[b] += res.results[core]["out_part"].astype(np.float32)
    return out


# revision 13
# speedup vs baseline: 1.3886x; 1.1026x over previous
"""Causal self-attention (B=2, T=2048, C=2048, H=16) on 8 TRN2 NeuronCores.

Sharding: data-parallel over batch (2) x tensor-parallel over heads (4 heads
per core). Each core computes its 4 heads' QKV projection, causal attention,
and a partial output projection (row-sharded W_proj); the host sums the 4
fp16 partial projections per batch element.

All matmuls are fp16 (fp8 DoubleRow on this hardware doubles contraction per
column, not columns per cycle, so any >=2-term compensated-fp8 scheme is no
faster than fp16 while 1-term fp8 misses the accuracy gate). Speed comes from
keeping the PE streaming 512-wide fp16 matmuls continuously:

  * AV is flipped vs the original: stationary = V block [k,d], moving =
    P^T [k, 512q] -> Y^T [d, q] accumulates in PSUM already transposed for
    the projection. No PE transposes, no narrow LDWEIGHTS-bound matmuls.
  * Softmax denominator: P^T tiles accumulate (fp16, DVE) into ptsum[k, q];
    one ones-vector matmul gives denom[1, q]; GpSimd broadcasts it to
    [128, q]; DVE reciprocal + one multiply normalize Y^T into fp16.
  * Software pipelining: the QKV-projection matmul groups of chunk j+1 and
    the output-projection groups of chunk j-1 are interleaved into the
    attention chains of chunk j, so the ACT-bound exp stream always has PE
    work running underneath it and the kernel stays PE-bound end to end.
"""

import os

import numpy as np

N_HEAD = 16
N_EMBD = 2048
B = 2
T = 2048
C = N_EMBD
D = C // N_HEAD  # 128
HPC = N_HEAD // 4  # heads per core = 4
N_CORES = 8
CT = C // 128  # 16 contraction tiles
TT = T // 128  # 16 t tiles
NCH = T // 512  # 4 chunks of 512

LAST_EXEC_NS = None

_CACHE = {}


def _build_nc():
    import concourse.bass as bass  # noqa: F401
    import concourse.tile as tile
    from concourse import bacc, mybir

    F32 = mybir.dt.float32
    F16 = mybir.dt.float16
    Exp = mybir.ActivationFunctionType.Exp
    Copy = mybir.ActivationFunctionType.Copy
    SCALE = 1.0 / float(np.sqrt(D))

    nc = bacc.Bacc("TRN2", target_bir_lowering=False, num_devices=N_CORES)

    xT_d = nc.dram_tensor("xT", [C, T], F16, kind="ExternalInput")
    wqk_d = nc.dram_tensor("wqk", [C, 8 * 128], F16, kind="ExternalInput")
    wv_d = nc.dram_tensor("wv", [C, 4 * 128], F16, kind="ExternalInput")
    wp_d = nc.dram_tensor("wp", [4 * 128, C], F16, kind="ExternalInput")
    out_d = nc.dram_tensor("out_part", [T, C], F16, kind="ExternalOutput")

    kk = np.arange(128)[:, None]
    qq = np.arange(512)[None, :]
    masks = np.stack(
        [(qq >= (128 * i + kk)).astype(np.float16) for i in range(4)]
    )  # [4, 128, 512]
    masks_d = nc.inline_tensor(np.ascontiguousarray(masks), name="diagmasks")
    ones128_d = nc.inline_tensor(np.ones((128, 1), dtype=np.float16), name="ones128")

    with tile.TileContext(nc) as tc:
        with (
            tc.tile_pool(name="singles", bufs=1) as singles,
            tc.tile_pool(name="xtp", bufs=32) as xtp,
            tc.tile_pool(name="ptp", bufs=6) as ptp,
            tc.tile_pool(name="ptsp", bufs=3) as ptsp,
            tc.tile_pool(name="rtp", bufs=2) as rtp,
            tc.tile_pool(name="rbp", bufs=2) as rbp,
            tc.tile_pool(name="ost", bufs=3) as ostp,
            tc.tile_pool(name="ps", bufs=4, space="PSUM") as ps,
            tc.tile_pool(name="pop", bufs=2, space="PSUM") as pop,
            tc.tile_pool(name="yps", bufs=2, space="PSUM") as yps,
        ):
            def load_x_chunk(tj):
                xt = []
                for c in range(CT):
                    xc = xtp.tile([128, 512], F16, tag="xt", name=f"xt{tj}_{c}")
                    nc.sync.dma_start(
                        out=xc,
                        in_=xT_d[
                            c * 128 : (c + 1) * 128, tj * 512 : (tj + 1) * 512
                        ],
                    )
                    xt.append(xc)
                return xt

            xt0 = load_x_chunk(0)
            # QK weights sliced per output col-tile: the first matmul group
            # only waits on one 256 KB tile (+ the chunk-0 x tiles).
            wqk_t = []
            for ct in range(8):
                w = singles.tile([128, CT, 128], F16, name=f"wqk{ct}")
                nc.sync.dma_start(
                    out=w,
                    in_=wqk_d[:, ct * 128 : (ct + 1) * 128].rearrange(
                        "(a p) n -> p a n", p=128
                    ),
                )
                wqk_t.append(w)

            # qkt: [d, coltile, t]; coltiles 0..3 = Q heads, 4..7 = K heads
            qkt_sb = singles.tile([128, 8, T], F16)
            # v: [kt-tile, head, d]
            vv_sb = singles.tile([128, TT, HPC, 128], F16)
            # y transposed: [d, head, t]
            yt_sb = singles.tile([128, HPC, T], F16)

            wv_sb = singles.tile([128, CT, 512], F16, name="wv_sb")
            wp_sb = singles.tile([128, HPC, C], F16, name="wp_sb")
            mask_sb = singles.tile([128, 4, 512], F16, name="mask_sb")
            ones128_sb = singles.tile([128, 1], F16, name="ones128_sb")

            def load_aux():
                nc.sync.dma_start(
                    out=wv_sb, in_=wv_d[:, :].rearrange("(a p) n -> p a n", p=128)
                )
                nc.sync.dma_start(
                    out=wp_sb, in_=wp_d[:, :].rearrange("(a p) n -> p a n", p=128)
                )
                nc.sync.dma_start(
                    out=mask_sb, in_=masks_d[:, :, :].rearrange("a p n -> p a n")
                )
                nc.sync.dma_start(out=ones128_sb, in_=ones128_d[:, :])

            # ---- emission pieces ----
            def qkv_pieces(tj, xt):
                pieces = []

                def mk_qk(ct):
                    def go():
                        pq = ps.tile([128, 512], F32, tag="ps", name=f"pq{tj}_{ct}")
                        for c in range(CT):
                            nc.tensor.matmul(
                                pq,
                                wqk_t[ct][:, c, :],
                                xt[c],
                                start=(c == 0),
                                stop=(c == CT - 1),
                            )
                        nc.scalar.activation(
                            out=qkt_sb[:, ct, tj * 512 : (tj + 1) * 512],
                            in_=pq,
                            func=Copy,
                        )

                    return go

                def mk_v(tt):
                    def go():
                        kt = tj * 4 + tt
                        pv = ps.tile([128, 512], F32, tag="ps", name=f"pv{kt}")
                        for c in range(CT):
                            nc.tensor.matmul(
                                pv,
                                xt[c][:, tt * 128 : (tt + 1) * 128],
                                wv_sb[:, c, :],
                                start=(c == 0),
                                stop=(c == CT - 1),
                            )
                        nc.scalar.activation(
                            out=vv_sb[:, kt, :, :],
                            in_=pv.rearrange("p (h d) -> p h d", h=HPC),
                            func=Copy,
                        )

                    return go

                for ct in range(8):
                    pieces.append(mk_qk(ct))
                for tt in range(4):
                    pieces.append(mk_v(tt))
                return pieces

            ot_cur = {}

            def proj_pieces(j):
                pieces = []

                def mk(tt, cc):
                    def go():
                        po = pop.tile([128, 512], F32, tag="po", name=f"po{tt}_{cc}")
                        for hd in range(HPC):
                            nc.tensor.matmul(
                                po,
                                yt_sb[:, hd, tt * 128 : (tt + 1) * 128],
                                wp_sb[:, hd, cc * 512 : (cc + 1) * 512],
                                start=(hd == 0),
                                stop=(hd == HPC - 1),
                            )
                        if cc == 0:
                            ot_cur[tt] = ostp.tile(
                                [128, C], F16, tag="ot", name=f"ot{tt}"
                            )
                        ot = ot_cur[tt]
                        if cc % 2 == 0:
                            nc.vector.tensor_copy(
                                out=ot[:, cc * 512 : (cc + 1) * 512], in_=po
                            )
                        else:
                            nc.scalar.activation(
                                out=ot[:, cc * 512 : (cc + 1) * 512], in_=po, func=Copy
                            )
                        if cc == 3:
                            nc.sync.dma_start(
                                out=out_d[tt * 128 : (tt + 1) * 128, :], in_=ot
                            )

                    return go

                for tt in range(4 * j, 4 * j + 4):
                    for cc in range(4):
                        pieces.append(mk(tt, cc))
                return pieces

            # ---- pipelined emission ----
            p0 = qkv_pieces(0, xt0)
            for p in p0[:8]:
                p()
            load_aux()
            for p in p0[8:]:
                p()

            prev_proj = []
            for j in range(NCH):
                if j < NCH - 1:
                    xt_next = load_x_chunk(j + 1)
                    nxt_pieces = qkv_pieces(j + 1, xt_next)
                else:
                    nxt_pieces = []
                # round-robin: proj of chunk j-1 and qkv of chunk j+1
                mixed = []
                a, b = iter(prev_proj), iter(nxt_pieces)
                while True:
                    pa, pb = next(a, None), next(b, None)
                    if pa is None and pb is None:
                        break
                    if pa is not None:
                        mixed.append(pa)
                    if pb is not None:
                        mixed.append(pb)
                filler = iter(mixed)

                nkt = 4 * j + 4
                for h in range(HPC):
                    yT = yps.tile([128, 512], F32, tag="y", name=f"yT{h}_{j}")
                    pts = ptsp.tile([128, 512], F16, tag="pts", name=f"pts{h}_{j}")
                    for kt in range(nkt):
                        di = kt - 4 * j
                        lo = 128 * di if di > 0 else 0
                        ss = ps.tile([128, 512], F32, tag="ps", name=f"ss{h}{j}{kt}")
                        nc.tensor.matmul(
                            ss[:, lo:],
                            qkt_sb[:, 4 + h, kt * 128 : (kt + 1) * 128],
                            qkt_sb[:, h, j * 512 + lo : (j + 1) * 512],
                            start=True,
                            stop=True,
                        )
                        pt = ptp.tile([128, 512], F16, tag="pt", name=f"pt{h}{j}{kt}")
                        nc.scalar.activation(
                            out=pt[:, lo:], in_=ss[:, lo:], func=Exp, scale=SCALE
                        )
                        if di >= 0:
                            nc.vector.tensor_mul(
                                pt[:, lo : lo + 128],
                                pt[:, lo : lo + 128],
                                mask_sb[:, di, lo : lo + 128],
                            )
                        if kt == 0:
                            nc.vector.tensor_copy(out=pts, in_=pt)
                        else:
                            nc.vector.tensor_add(
                                pts[:, lo:], pts[:, lo:], pt[:, lo:]
                            )
                        nc.tensor.matmul(
                            yT[:, lo:],
                            vv_sb[:, kt, h, :],
                            pt[:, lo:],
                            start=(kt == 0),
                            stop=(kt == nkt - 1),
                            skip_group_check=True,
                        )
                        nf = next(filler, None)
                        if nf is not None:
                            nf()
                    # denominator -> broadcast -> reciprocal -> normalize
                    dT = ps.tile([1, 512], F32, tag="ps", name=f"dT{h}_{j}")
                    nc.tensor.matmul(dT, ones128_sb, pts, start=True, stop=True)
                    dTs = rtp.tile([1, 512], F32, tag="dts", name=f"dTs{h}_{j}")
                    nc.scalar.activation(out=dTs, in_=dT, func=Copy)
                    rb32 = rbp.tile([128, 512], F32, tag="rb32", name=f"rb32_{h}{j}")
                    nc.gpsimd.partition_broadcast(rb32, dTs)
                    rb16 = rbp.tile([128, 512], F32, tag="rb16", name=f"rb16_{h}{j}")
                    nc.vector.reciprocal_approx_fast(rb16, rb32)
                    nc.vector.tensor_mul(
                        yt_sb[:, h, j * 512 : (j + 1) * 512], yT, rb16
                    )
                for rem in filler:
                    rem()
                prev_proj = proj_pieces(j)
            for p in prev_proj:
                p()

    nc.compile()
    return nc


def _get_nc():
    if "nc" not in _CACHE:
        _CACHE["nc"] = _build_nc()
    return _CACHE["nc"]


def kernel(x, W_attn, W_proj):
    global LAST_EXEC_NS
    from concourse.bass_utils import run_bass_kernel_spmd

    x = np.asarray(x)
    W_attn = np.asarray(W_attn)
    W_proj = np.asarray(W_proj)

    in_maps = []
    for core in range(N_CORES):
        b, g = divmod(core, 4)
        heads = range(4 * g, 4 * g + 4)
        xT = np.ascontiguousarray(x[b].T).astype(np.float16)
        wqk = np.concatenate(
            [W_attn[:, h * D : (h + 1) * D] for h in heads]
            + [W_attn[:, C + h * D : C + (h + 1) * D] for h in heads],
            axis=1,
        ).astype(np.float16)
        wv = np.concatenate(
            [W_attn[:, 2 * C + h * D : 2 * C + (h + 1) * D] for h in heads], axis=1
        ).astype(np.float16)
        wp = W_proj[4 * g * D : 4 * (g + 1) * D, :].astype(np.float16)
        in_maps.append({"xT": xT, "wqk": wqk, "wv": wv, "wp": wp})

    nc = _get_nc()
    res = run_bass_kernel_spmd(
        nc,
        in_maps,
        list(range(N_CORES)),
        trace=bool(os.environ.get("KERNEL_TRACE")),
    )
    LAST_EXEC_NS = res.exec_time_ns

    out = np.zeros((B, T, C), dtype=np.float32)
    for core in range(N_CORES):
        b = core // 4
        out[b] += res.results[core]["out_part"].astype(np.float32)
    return out


# revision 14
# speedup vs baseline: 1.3997x; 1.0079x over previous
"""Causal self-attention (B=2, T=2048, C=2048, H=16) on 8 TRN2 NeuronCores.

Sharding: data-parallel over batch (2) x tensor-parallel over heads (4 heads
per core). Each core computes its 4 heads' QKV projection, causal attention,
and a partial output projection (row-sharded W_proj); the host sums the 4
fp16 partial projections per batch element.

All matmuls are fp16 (fp8 DoubleRow on this hardware doubles contraction per
column, not columns per cycle, so any >=2-term compensated-fp8 scheme is no
faster than fp16 while 1-term fp8 misses the accuracy gate). Speed comes from
keeping the PE streaming 512-wide fp16 matmuls continuously:

  * AV is flipped vs the original: stationary = V block [k,d], moving =
    P^T [k, 512q] -> Y^T [d, q] accumulates in PSUM already transposed for
    the projection. No PE transposes, no narrow LDWEIGHTS-bound matmuls.
  * Softmax denominator: P^T tiles accumulate (fp16, DVE) into ptsum[k, q];
    one ones-vector matmul gives denom[1, q]; GpSimd broadcasts it to
    [128, q]; DVE reciprocal + one multiply normalize Y^T into fp16.
  * Software pipelining: the QKV-projection matmul groups of chunk j+1 and
    the output-projection groups of chunk j-1 are interleaved into the
    attention chains of chunk j, so the ACT-bound exp stream always has PE
    work running underneath it and the kernel stays PE-bound end to end.
"""

import os

import numpy as np

N_HEAD = 16
N_EMBD = 2048
B = 2
T = 2048
C = N_EMBD
D = C // N_HEAD  # 128
HPC = N_HEAD // 4  # heads per core = 4
N_CORES = 8
CT = C // 128  # 16 contraction tiles
TT = T // 128  # 16 t tiles
NCH = T // 512  # 4 chunks of 512

LAST_EXEC_NS = None

_CACHE = {}


def _build_nc():
    import concourse.bass as bass  # noqa: F401
    import concourse.tile as tile
    from concourse import bacc, mybir

    F32 = mybir.dt.float32
    F16 = mybir.dt.float16
    Exp = mybir.ActivationFunctionType.Exp
    Copy = mybir.ActivationFunctionType.Copy
    SCALE = 1.0 / float(np.sqrt(D))

    nc = bacc.Bacc("TRN2", target_bir_lowering=False, num_devices=N_CORES)

    xT_d = nc.dram_tensor("xT", [C, T], F16, kind="ExternalInput")
    wqk_d = nc.dram_tensor("wqk", [C, 8 * 128], F16, kind="ExternalInput")
    wv_d = nc.dram_tensor("wv", [C, 4 * 128], F16, kind="ExternalInput")
    wp_d = nc.dram_tensor("wp", [4 * 128, C], F16, kind="ExternalInput")
    out_d = nc.dram_tensor("out_part", [T, C], F16, kind="ExternalOutput")

    kk = np.arange(128)[:, None]
    qq = np.arange(512)[None, :]
    masks = np.stack(
        [(qq >= (128 * i + kk)).astype(np.float16) for i in range(4)]
    )  # [4, 128, 512]
    masks_d = nc.inline_tensor(np.ascontiguousarray(masks), name="diagmasks")
    ones128_d = nc.inline_tensor(np.ones((128, 1), dtype=np.float16), name="ones128")

    with tile.TileContext(nc) as tc:
        with (
            tc.tile_pool(name="singles", bufs=1) as singles,
            tc.tile_pool(name="xtp", bufs=32) as xtp,
            tc.tile_pool(name="ptp", bufs=6) as ptp,
            tc.tile_pool(name="ptsp", bufs=3) as ptsp,
            tc.tile_pool(name="rtp", bufs=2) as rtp,
            tc.tile_pool(name="rbp", bufs=2) as rbp,
            tc.tile_pool(name="ost", bufs=3) as ostp,
            tc.tile_pool(name="ps", bufs=4, space="PSUM") as ps,
            tc.tile_pool(name="pop", bufs=2, space="PSUM") as pop,
            tc.tile_pool(name="yps", bufs=2, space="PSUM") as yps,
        ):
            def load_x_chunk(tj):
                xt = []
                for c in range(CT):
                    xc = xtp.tile([128, 512], F16, tag="xt", name=f"xt{tj}_{c}")
                    nc.sync.dma_start(
                        out=xc,
                        in_=xT_d[
                            c * 128 : (c + 1) * 128, tj * 512 : (tj + 1) * 512
                        ],
                    )
                    xt.append(xc)
                return xt

            # QK weights sliced per output col-tile, DMA-interleaved with the
            # chunk-0 x tiles so matmul group ct never waits on later groups'
            # traffic.
            wqk_t = [
                singles.tile([128, CT, 128], F16, name=f"wqk{ct}") for ct in range(8)
            ]

            def load_wqk(ct):
                nc.sync.dma_start(
                    out=wqk_t[ct],
                    in_=wqk_d[:, ct * 128 : (ct + 1) * 128].rearrange(
                        "(a p) n -> p a n", p=128
                    ),
                )

            load_wqk(0)
            xt0 = []
            for c in range(CT):
                xc = xtp.tile([128, 512], F16, tag="xt", name=f"xt0_{c}")
                nc.sync.dma_start(
                    out=xc, in_=xT_d[c * 128 : (c + 1) * 128, 0:512]
                )
                xt0.append(xc)
                if c % 2 == 1 and c // 2 + 1 < 8:
                    load_wqk(c // 2 + 1)

            # qkt: [d, coltile, t]; coltiles 0..3 = Q heads, 4..7 = K heads
            qkt_sb = singles.tile([128, 8, T], F16)
            # v: [kt-tile, head, d]
            vv_sb = singles.tile([128, TT, HPC, 128], F16)
            # y transposed: [d, head, t]
            yt_sb = singles.tile([128, HPC, T], F16)

            wv_sb = singles.tile([128, CT, 512], F16, name="wv_sb")
            wp_sb = singles.tile([128, HPC, C], F16, name="wp_sb")
            mask_sb = singles.tile([128, 4, 512], F16, name="mask_sb")
            ones128_sb = singles.tile([128, 1], F16, name="ones128_sb")

            def load_aux():
                nc.sync.dma_start(
                    out=wv_sb, in_=wv_d[:, :].rearrange("(a p) n -> p a n", p=128)
                )
                nc.sync.dma_start(
                    out=wp_sb, in_=wp_d[:, :].rearrange("(a p) n -> p a n", p=128)
                )
                nc.sync.dma_start(
                    out=mask_sb, in_=masks_d[:, :, :].rearrange("a p n -> p a n")
                )
                nc.sync.dma_start(out=ones128_sb, in_=ones128_d[:, :])

            # ---- emission pieces ----
            def qkv_pieces(tj, xt):
                pieces = []

                def mk_qk(ct):
                    def go():
                        pq = ps.tile([128, 512], F32, tag="ps", name=f"pq{tj}_{ct}")
                        for c in range(CT):
                            nc.tensor.matmul(
                                pq,
                                wqk_t[ct][:, c, :],
                                xt[c],
                                start=(c == 0),
                                stop=(c == CT - 1),
                            )
                        nc.scalar.activation(
                            out=qkt_sb[:, ct, tj * 512 : (tj + 1) * 512],
                            in_=pq,
                            func=Copy,
                        )

                    return go

                def mk_v(tt):
                    def go():
                        kt = tj * 4 + tt
                        pv = ps.tile([128, 512], F32, tag="ps", name=f"pv{kt}")
                        for c in range(CT):
                            nc.tensor.matmul(
                                pv,
                                xt[c][:, tt * 128 : (tt + 1) * 128],
                                wv_sb[:, c, :],
                                start=(c == 0),
                                stop=(c == CT - 1),
                            )
                        nc.scalar.activation(
                            out=vv_sb[:, kt, :, :],
                            in_=pv.rearrange("p (h d) -> p h d", h=HPC),
                            func=Copy,
                        )

                    return go

                for ct in range(8):
                    pieces.append(mk_qk(ct))
                for tt in range(4):
                    pieces.append(mk_v(tt))
                return pieces

            ot_cur = {}

            def proj_pieces(j):
                pieces = []

                def mk(tt, cc):
                    def go():
                        po = pop.tile([128, 512], F32, tag="po", name=f"po{tt}_{cc}")
                        for hd in range(HPC):
                            nc.tensor.matmul(
                                po,
                                yt_sb[:, hd, tt * 128 : (tt + 1) * 128],
                                wp_sb[:, hd, cc * 512 : (cc + 1) * 512],
                                start=(hd == 0),
                                stop=(hd == HPC - 1),
                            )
                        if cc == 0:
                            ot_cur[tt] = ostp.tile(
                                [128, C], F16, tag="ot", name=f"ot{tt}"
                            )
                        ot = ot_cur[tt]
                        if cc % 2 == 0:
                            nc.vector.tensor_copy(
                                out=ot[:, cc * 512 : (cc + 1) * 512], in_=po
                            )
                        else:
                            nc.scalar.activation(
                                out=ot[:, cc * 512 : (cc + 1) * 512], in_=po, func=Copy
                            )
                        if cc == 3:
                            nc.sync.dma_start(
                                out=out_d[tt * 128 : (tt + 1) * 128, :], in_=ot
                            )

                    return go

                for tt in range(4 * j, 4 * j + 4):
                    for cc in range(4):
                        pieces.append(mk(tt, cc))
                return pieces

            # ---- pipelined emission ----
            p0 = qkv_pieces(0, xt0)
            for p in p0[:8]:
                p()
            load_aux()
            for p in p0[8:]:
                p()

            prev_proj = []
            for j in range(NCH):
                if j < NCH - 1:
                    xt_next = load_x_chunk(j + 1)
                    nxt_pieces = qkv_pieces(j + 1, xt_next)
                else:
                    nxt_pieces = []
                # round-robin: proj of chunk j-1 and qkv of chunk j+1
                mixed = []
                a, b = iter(prev_proj), iter(nxt_pieces)
                while True:
                    pa, pb = next(a, None), next(b, None)
                    if pa is None and pb is None:
                        break
                    if pa is not None:
                        mixed.append(pa)
                    if pb is not None:
                        mixed.append(pb)
                filler = iter(mixed)
                nkt = 4 * j + 4
                slots = HPC * nkt
                stride = max(1, slots // max(1, len(mixed)))
                slot = 0
                for h in range(HPC):
                    yT = yps.tile([128, 512], F32, tag="y", name=f"yT{h}_{j}")
                    pts = ptsp.tile([128, 512], F16, tag="pts", name=f"pts{h}_{j}")
                    for kt in range(nkt):
                        di = kt - 4 * j
                        lo = 128 * di if di > 0 else 0
                        ss = ps.tile([128, 512], F32, tag="ps", name=f"ss{h}{j}{kt}")
                        nc.tensor.matmul(
                            ss[:, lo:],
                            qkt_sb[:, 4 + h, kt * 128 : (kt + 1) * 128],
                            qkt_sb[:, h, j * 512 + lo : (j + 1) * 512],
                            start=True,
                            stop=True,
                        )
                        pt = ptp.tile([128, 512], F16, tag="pt", name=f"pt{h}{j}{kt}")
                        nc.scalar.activation(
                            out=pt[:, lo:], in_=ss[:, lo:], func=Exp, scale=SCALE
                        )
                        if di >= 0:
                            nc.vector.tensor_mul(
                                pt[:, lo : lo + 128],
                                pt[:, lo : lo + 128],
                                mask_sb[:, di, lo : lo + 128],
                            )
                        if kt == 0:
                            nc.vector.tensor_copy(out=pts, in_=pt)
                        else:
                            nc.vector.tensor_add(
                                pts[:, lo:], pts[:, lo:], pt[:, lo:]
                            )
                        nc.tensor.matmul(
                            yT[:, lo:],
                            vv_sb[:, kt, h, :],
                            pt[:, lo:],
                            start=(kt == 0),
                            stop=(kt == nkt - 1),
                            skip_group_check=True,
                        )
                        slot += 1
                        if slot % stride == 0:
                            nf = next(filler, None)
                            if nf is not None:
                                nf()
                    # denominator -> broadcast -> reciprocal -> normalize
                    dT = ps.tile([1, 512], F32, tag="ps", name=f"dT{h}_{j}")
                    nc.tensor.matmul(dT, ones128_sb, pts, start=True, stop=True)
                    dTs = rtp.tile([1, 512], F32, tag="dts", name=f"dTs{h}_{j}")
                    nc.scalar.activation(out=dTs, in_=dT, func=Copy)
                    rb32 = rbp.tile([128, 512], F32, tag="rb32", name=f"rb32_{h}{j}")
                    nc.gpsimd.partition_broadcast(rb32, dTs)
                    rb16 = rbp.tile([128, 512], F32, tag="rb16", name=f"rb16_{h}{j}")
                    nc.vector.reciprocal_approx_fast(rb16, rb32)
                    nc.vector.tensor_mul(
                        yt_sb[:, h, j * 512 : (j + 1) * 512], yT, rb16
                    )
                for rem in filler:
                    rem()
                prev_proj = proj_pieces(j)
            for p in prev_proj:
                p()

    nc.compile()
    return nc


def _get_nc():
    if "nc" not in _CACHE:
        _CACHE["nc"] = _build_nc()
    return _CACHE["nc"]


def kernel(x, W_attn, W_proj):
    global LAST_EXEC_NS
    from concourse.bass_utils import run_bass_kernel_spmd

    x = np.asarray(x)
    W_attn = np.asarray(W_attn)
    W_proj = np.asarray(W_proj)

    in_maps = []
    for core in range(N_CORES):
        b, g = divmod(core, 4)
        heads = range(4 * g, 4 * g + 4)
        xT = np.ascontiguousarray(x[b].T).astype(np.float16)
        wqk = np.concatenate(
            [W_attn[:, h * D : (h + 1) * D] for h in heads]
            + [W_attn[:, C + h * D : C + (h + 1) * D] for h in heads],
            axis=1,
        ).astype(np.float16)
        wv = np.concatenate(
            [W_attn[:, 2 * C + h * D : 2 * C + (h + 1) * D] for h in heads], axis=1
        ).astype(np.float16)
        wp = W_proj[4 * g * D : 4 * (g + 1) * D, :].astype(np.float16)
        in_maps.append({"xT": xT, "wqk": wqk, "wv": wv, "wp": wp})

    nc = _get_nc()
    res = run_bass_kernel_spmd(
        nc,
        in_maps,
        list(range(N_CORES)),
        trace=bool(os.environ.get("KERNEL_TRACE")),
    )
    LAST_EXEC_NS = res.exec_time_ns

    out = np.zeros((B, T, C), dtype=np.float32)
    for core in range(N_CORES):
        b = core // 4
        out[b] += res.results[core]["out_part"].astype(np.float32)
    return out
